# revision 18
# baseline (speedup 1.0000x reference)
"""MEGADecoder forward pass as a Bass/Tile kernel on 8 TRN2 NeuronCores. v2.

Sharding: sequence-parallel, 512 rows/core, params replicated. Single-head
full attention via AllGather of (K.T, V) in bf16, split into two collectives
(K.T first, V second) so gathers overlap the Q / R_EMA' / f / i projections.

All matmul operands are bf16 (full PE rate, half the DMA/collective bytes of
f32r); PSUM accumulation is f32. Weights are pre-shuffled on the host so every
weight DMA is contiguous per partition line.

Layout: activations feature-major [128 part x FC chunks x 512 seq free]; every
GEMM is a chain of 128x128x512 PE matmuls, no transposes. Softmax runs without
max-subtraction; denominator from a ones-vector matmul accumulated over
j-chunks.
"""

import numpy as np

SEQ = 4096
D = 1024
NCORES = 8
S = SEQ // NCORES  # 512 rows per core
P = 128
FC = D // P  # 8 feature chunks
NJ = SEQ // P  # 32 j-chunks
ATT_SCALE = 1.0 / float(np.sqrt(np.float32(D)))

KT_ELEMS = D * S
V_ELEMS = S * D

_CACHE = {}


def _build_bass(with_collective=True, reps=1, split_kv=False):
    import concourse.bacc as bacc
    import concourse.tile as tile
    import concourse.mybir as mybir

    f32 = mybir.dt.float32
    bf16 = mybir.dt.bfloat16
    fmm = bf16
    AF = mybir.ActivationFunctionType

    nc = bacc.Bacc(None, target_bir_lowering=False, num_devices=NCORES)
    mm = nc.tensor.matmul

    # ---- DRAM I/O (host pre-shuffled layouts) ----
    # rt: [P, FC*(S+1)] so the load is one contiguous DMA per partition line.
    rt = nc.dram_tensor("rt", [P, FC * (S + 1)], fmm, kind="ExternalInput")
    # big weights as [halves, P, 2FC or FC, 512]-style blocks, contiguous.
    w_in = {}
    for name, shp in [
        ("wa", [FC, P, 2 * FC * P]),   # per out-chunk blocks
        ("wd", [FC, P, 2 * FC * P]),
        ("wz", [2, P, FC * 4 * P]),    # per half blocks
        ("wq", [2, P, FC * 4 * P]),
        ("wk", [2, P, FC * 4 * P]),
        ("wv", [2, P, FC * 4 * P]),
        ("wema", [2, P, FC * 4 * P]),
        ("wf", [2, P, FC * 4 * P]),
        ("wzat", [2, P, FC * 4 * P]),
        ("wi", [P, FC]),
        ("wfin", [P, FC]),
    ]:
        w_in[name] = nc.dram_tensor(name, shp, fmm, kind="ExternalInput")
    # biases packed [10, D]: rows alpha,delta,z,q(pre-scaled),k,v,ema,f,zat,i
    biases = nc.dram_tensor("biases", [10, D], f32, kind="ExternalInput")
    out = nc.dram_tensor("out", [S, 1], f32, kind="ExternalOutput")

    with tile.TileContext(nc) as tc, \
         tc.tile_pool(name="consts", bufs=1) as consts, \
         tc.tile_pool(name="dram", bufs=1, space="DRAM") as dram, \
         tc.tile_pool(name="bigd", bufs=2) as bigd, \
         tc.tile_pool(name="big", bufs=1) as big:

        bsb = consts.tile([P, 10, FC], f32)
        nc.sync.dma_start(out=bsb, in_=biases.ap().rearrange("b (c p) -> p b c", p=P))
        ones_f32 = consts.tile([P, 1], f32)
        nc.vector.memset(ones_f32, 1.0)
        ones_col = consts.tile([P, 1], fmm)
        nc.scalar.copy(ones_col, ones_f32)
        bv_b = consts.tile([P, D], f32)
        nc.sync.dma_start(out=bv_b, in_=biases.ap()[5:6, :].partition_broadcast(P))

        def bias_ap(row, chunk):
            return bsb[:, row, chunk:chunk + 1]

        for _rep in range(reps):
          # Per-rep DRAM buffers: a Shared (collective-output) tensor may only
          # be written by a single instruction, so reps can't share them.
          row_bounce = dram.tile([2, S], f32, tag=f"rb{_rep}")
          if split_kv:
              kt_in = dram.tile([KT_ELEMS], fmm, tag=f"kti{_rep}")
              kt_out = dram.tile([NCORES, KT_ELEMS], fmm, addr_space="Shared",
                                 tag=f"kto{_rep}")
              v_in = dram.tile([V_ELEMS], fmm, tag=f"vi{_rep}")
              v_out = dram.tile([NCORES, V_ELEMS], fmm, addr_space="Shared",
                                tag=f"vo{_rep}")
          else:
              kv_in = dram.tile([KT_ELEMS + V_ELEMS], fmm, tag=f"kvi{_rep}")
              kv_out = dram.tile([NCORES, KT_ELEMS + V_ELEMS], fmm,
                                 addr_space="Shared", tag=f"kvo{_rep}")
              kt_in = kv_in[0:KT_ELEMS]
              v_in = kv_in[KT_ELEMS:]
          # Preload every weight consumed AFTER the collectives are issued: a
          # DMA issued after a collective in program order stalls until the
          # collective completes, so post-collective phases must not DMA.
          with tc.tile_pool(name="wpre", bufs=1) as wpre_pool:
            wpre = {}
            for wname in ("wq", "wema", "wf"):
                for half in range(2):
                    t = wpre_pool.tile([P, FC, 4 * P], fmm,
                                       tag=f"{wname}{half}")
                    nc.sync.dma_start(
                        out=t,
                        in_=w_in[wname].ap()[half].rearrange(
                            "p (c o) -> p c o", c=FC))
                    wpre[(wname, half)] = t
            wi_sb = wpre_pool.tile([P, FC], fmm, tag="wi")
            nc.sync.dma_start(out=wi_sb, in_=w_in["wi"].ap())
            wfin_sb = wpre_pool.tile([P, FC], fmm, tag="wfin")
            nc.sync.dma_start(out=wfin_sb, in_=w_in["wfin"].ap())
            # whole-kernel resident activations
            rema = bigd.tile([P, FC, S], fmm, tag="rema")
            z = bigd.tile([P, FC, S], fmm, tag="z")
            qT = bigd.tile([P, FC, S], fmm, tag="qT")
            remap = bigd.tile([P, FC, S], fmm, tag="remap")
            fT = big.tile([P, FC, S], f32, tag="fT")
            zatp = big.tile([P, FC, S], fmm, tag="zatp")
            ib = big.tile([P, S], f32, tag="ib")


            # ---------------- Phase 1: R_EMA ----------------
            with tc.tile_pool(name="p_rt", bufs=1) as p_rt, \
                 tc.tile_pool(name="p1w", bufs=3) as p1w, \
                 tc.tile_pool(name="p1ps", bufs=2, space="PSUM") as p1ps, \
                 tc.tile_pool(name="p1t", bufs=2) as p1t:
                rt_sb = p_rt.tile([P, FC, S + 1], fmm)
                rt_src = rt.ap().rearrange("p (c s) -> p c s", c=FC)
                nc.sync.dma_start(out=rt_sb[:, 0:2, :], in_=rt_src[:, 0:2, :])
                nc.sync.dma_start(out=rt_sb[:, 2:FC, :], in_=rt_src[:, 2:FC, :])
                for ot in range(FC):
                    wa_t = p1w.tile([P, 2 * FC, P], fmm, tag="wa")
                    nc.sync.dma_start(
                        out=wa_t,
                        in_=w_in["wa"].ap()[ot].rearrange("p (c o) -> p c o", o=P))
                    wd_t = p1w.tile([P, 2 * FC, P], fmm, tag="wd")
                    nc.sync.dma_start(
                        out=wd_t,
                        in_=w_in["wd"].ap()[ot].rearrange("p (c o) -> p c o", o=P))
                    ps_a = p1ps.tile([P, S], f32, tag="psa")
                    ps_d = p1ps.tile([P, S], f32, tag="psd")
                    for ch in range(FC):
                        mm(ps_a, wa_t[:, ch, :], rt_sb[:, ch, 0:S],
                           start=(ch == 0), stop=False)
                        mm(ps_d, wd_t[:, ch, :], rt_sb[:, ch, 0:S],
                           start=(ch == 0), stop=False)
                    for ch in range(FC):
                        mm(ps_a, wa_t[:, FC + ch, :], rt_sb[:, ch, 1:S + 1],
                           start=False, stop=(ch == FC - 1))
                        mm(ps_d, wd_t[:, FC + ch, :], rt_sb[:, ch, 1:S + 1],
                           start=False, stop=(ch == FC - 1))
                    alpha_t = p1t.tile([P, S], f32, tag="alpha")
                    nc.scalar.activation(alpha_t, ps_a, AF.Tanh,
                                         bias=bias_ap(0, ot), scale=1.0)
                    delta_t = p1t.tile([P, S], f32, tag="delta")
                    nc.scalar.activation(delta_t, ps_d, AF.Tanh,
                                         bias=bias_ap(1, ot), scale=1.0)
                    # rema = t1 + alpha*(r_t - t1), t1 = delta*r_prev
                    t1 = p1t.tile([P, S], f32, tag="t1")
                    nc.vector.tensor_mul(t1, delta_t, rt_sb[:, ot, 0:S])
                    t2 = p1t.tile([P, S], f32, tag="t2")
                    nc.vector.tensor_sub(t2, rt_sb[:, ot, 1:S + 1], t1)
                    t3 = p1t.tile([P, S], f32, tag="t3")
                    nc.vector.tensor_mul(t3, alpha_t, t2)
                    nc.vector.tensor_add(rema[:, ot, :], t3, t1)

            # ---------------- Phase 2: Z, K.T, V (collectives), Q.T ----------
            with tc.tile_pool(name="p2w", bufs=3) as p2w, \
                 tc.tile_pool(name="p2ps", bufs=4, space="PSUM") as p2ps, \
                 tc.tile_pool(name="p_kv", bufs=1) as p_kv:
                def proj(w_name, rhs_src, out_tile, func, bias_row, scale=1.0,
                         chunk_dma=None, preloaded=False):
                    for half in range(2):
                        if preloaded:
                            w_t = wpre[(w_name, half)]
                        else:
                            w_t = p2w.tile([P, FC, 4 * P], fmm, tag="w")
                            nc.sync.dma_start(
                                out=w_t,
                                in_=w_in[w_name].ap()[half].rearrange(
                                    "p (c o) -> p c o", c=FC))
                        for sub in range(4):
                            ot = half * 4 + sub
                            ow = slice(sub * P, (sub + 1) * P)
                            ps = p2ps.tile([P, S], f32, tag="ps")
                            for ch in range(FC):
                                mm(ps, w_t[:, ch, ow], rhs_src[:, ch, :],
                                   start=(ch == 0), stop=(ch == FC - 1))
                            nc.scalar.activation(out_tile[:, ot, :], ps, func,
                                                 bias=bias_ap(bias_row, ot),
                                                 scale=scale)
                            if chunk_dma is not None:
                                chunk_dma(ot, out_tile)

                proj("wz", rema, z, AF.Silu, 2)

                # K.T chunks: DMA each chunk to kt_in as soon as it's ready.
                ktS = p_kv.tile([P, FC, S], fmm)

                def kt_dma(ot, out_tile):
                    with tc.high_priority():
                        nc.sync.dma_start(
                            out=kt_in[ot * P * S:(ot + 1) * P * S].rearrange(
                                "(p s) -> p s", p=P),
                            in_=out_tile[:, ot, :])

                proj("wk", z, ktS, AF.Identity, 4, chunk_dma=kt_dma)

                # V seq-major: V[s, o] = sum_d Z.T[d, s] Wv.T[d, o] (+ bv)
                for half in range(2):
                    osl = slice(half * 4 * P, (half + 1) * 4 * P)
                    wv_t = p2w.tile([P, FC, 4 * P], fmm, tag="w")
                    nc.sync.dma_start(
                        out=wv_t,
                        in_=w_in["wv"].ap()[half].rearrange("p (c o) -> p c o", c=FC))
                    for st in range(4):
                        ssl = slice(st * P, (st + 1) * P)
                        ps = p2ps.tile([P, 4 * P], f32, tag="ps")
                        for ch in range(FC):
                            mm(ps, z[:, ch, ssl], wv_t[:, ch, :],
                               start=(ch == 0), stop=(ch == FC - 1))
                        v_sb = p_kv.tile([P, 4 * P], fmm, tag="vsb")
                        nc.vector.tensor_add(v_sb, ps, bv_b[:, osl])
                        with tc.high_priority():
                            nc.sync.dma_start(
                                out=v_in[st * P * D:(st + 1) * P * D].rearrange(
                                    "(p o) -> p o", p=P)[:, osl],
                                in_=v_sb)

                if with_collective:
                    if split_kv:
                        with tc.high_priority():
                            nc.gpsimd.collective_compute(
                                "AllGather", mybir.AluOpType.bypass,
                                replica_groups=[list(range(NCORES))],
                                ins=[kt_in[:].opt()], outs=[kt_out[:].opt()],
                            )
                            nc.gpsimd.collective_compute(
                                "AllGather", mybir.AluOpType.bypass,
                                replica_groups=[list(range(NCORES))],
                                ins=[v_in[:].opt()], outs=[v_out[:].opt()],
                            )
                    else:
                        with tc.high_priority():
                            nc.gpsimd.collective_compute(
                                "AllGather", mybir.AluOpType.bypass,
                                replica_groups=[list(range(NCORES))],
                                ins=[kv_in[:].opt()], outs=[kv_out[:].opt()],
                            )

                # Q.T after the gathers are issued — overlaps them
                # (weights preloaded, so no post-collective DMA stall).
                proj("wq", z, qT, AF.Identity, 3, scale=ATT_SCALE,
                     preloaded=True)

            # ---------------- Phase 4: R_EMA', f, i ----------------
            with tc.tile_pool(name="p4ps", bufs=2, space="PSUM") as p4ps, \
                 tc.tile_pool(name="p4psi", bufs=1, space="PSUM") as p4psi, \
                 tc.tile_pool(name="p4t", bufs=1) as p4t:
                def proj4(w_name, rhs_src, out_tile, func, bias_row):
                    for half in range(2):
                        w_t = wpre[(w_name, half)]
                        for sub in range(4):
                            ot = half * 4 + sub
                            ow = slice(sub * P, (sub + 1) * P)
                            ps = p4ps.tile([P, S], f32, tag="ps")
                            for ch in range(FC):
                                mm(ps, w_t[:, ch, ow], rhs_src[:, ch, :],
                                   start=(ch == 0), stop=(ch == FC - 1))
                            nc.scalar.activation(out_tile[:, ot, :], ps, func,
                                                 bias=bias_ap(bias_row, ot),
                                                 scale=1.0)

                proj4("wema", rema, remap, AF.Identity, 6)
                proj4("wf", remap, fT, AF.Sigmoid, 7)

                ps_i = p4psi.tile([1, S], f32)
                for ch in range(FC):
                    mm(ps_i, wi_sb[:, ch:ch + 1], rema[:, ch, :],
                       start=(ch == 0), stop=(ch == FC - 1))
                i_row = p4t.tile([1, S], f32, tag="irow")
                nc.scalar.activation(i_row, ps_i, AF.Tanh,
                                     bias=bsb[0:1, 9, 0:1], scale=1.0)
                nc.sync.dma_start(out=row_bounce[0:1, :], in_=i_row)
                nc.sync.dma_start(out=ib,
                                  in_=row_bounce[0:1, :].partition_broadcast(P))


            # ---------------- Phase 5: attention ----------------
            with tc.tile_pool(name="p_pt", bufs=1) as p_pt, \
                 tc.tile_pool(name="p_rl", bufs=1) as p_rl:
                pt = p_pt.tile([P, NJ, S], fmm)
                rl_b = p_rl.tile([P, S], f32, tag="rlb")
                # --- 5A: scores.T + exp + denominator ---
                with tc.tile_pool(name="p5w", bufs=2) as p5w, \
                     tc.tile_pool(name="p5ps", bufs=2, space="PSUM") as p5ps, \
                     tc.tile_pool(name="p5lps", bufs=1, space="PSUM") as p5lps:
                    l_ps = p5lps.tile([1, S], f32)
                    for r in range(NCORES):
                        if split_kv:
                            kt_src = kt_out[r] if with_collective else kt_in[:]
                        else:
                            kt_src = (kv_out[r, 0:KT_ELEMS] if with_collective
                                      else kt_in[:])
                        kt_r = p5w.tile([P, FC, S], fmm, tag="ktr")
                        nc.sync.dma_start(
                            out=kt_r,
                            in_=kt_src.rearrange("(c p s) -> p c s", p=P, s=S))
                        for cl in range(4):
                            jc = r * 4 + cl
                            jsl = slice(cl * P, (cl + 1) * P)
                            s_ps = p5ps.tile([P, S], f32, tag="sps")
                            for ch in range(FC):
                                mm(s_ps, kt_r[:, ch, jsl], qT[:, ch, :],
                                   start=(ch == 0), stop=(ch == FC - 1))
                            nc.scalar.activation(pt[:, jc, :], s_ps, AF.Exp,
                                                 bias=0.0, scale=1.0)
                            mm(l_ps, ones_col, pt[:, jc, :],
                               start=(jc == 0), stop=(jc == NJ - 1))
                    l_row = p_rl.tile([1, S], f32, tag="lrow")
                    nc.vector.reciprocal(l_row, l_ps)
                    nc.sync.dma_start(out=row_bounce[1:2, :], in_=l_row)
                    nc.sync.dma_start(
                        out=rl_b, in_=row_bounce[1:2, :].partition_broadcast(P))

                # --- 5B: Zat.T ---
                with tc.tile_pool(name="p5v", bufs=6) as p5v, \
                     tc.tile_pool(name="pvps", bufs=1, space="PSUM") as pvps:
                    zat_ps = []
                    for i in range(FC):
                        zp = pvps.tile([P, S], f32, tag=f"zat{i}", name=f"zat{i}")
                        zat_ps.append(zp)
                    for jc in range(NJ):
                        r, cl = jc // 4, jc % 4
                        if split_kv:
                            v_src = v_out[r] if with_collective else v_in[:]
                        else:
                            v_src = (kv_out[r, KT_ELEMS:] if with_collective
                                     else kv_in[KT_ELEMS:])
                        v_ch = p5v.tile([P, D], fmm, tag="vch")
                        nc.sync.dma_start(
                            out=v_ch,
                            in_=v_src.rearrange("(t p o) -> t p o", p=P, o=D)[cl])
                        for ot in range(FC):
                            mm(zat_ps[ot], v_ch[:, ot * P:(ot + 1) * P],
                               pt[:, jc, :],
                               start=(jc == 0), stop=(jc == NJ - 1))
                    # zatp = f * (zat/l)
                    for ot in range(FC):
                        nc.vector.tensor_mul(zat_ps[ot], zat_ps[ot], rl_b)
                        nc.vector.tensor_mul(zatp[:, ot, :], zat_ps[ot],
                                             fT[:, ot, :])

            # ---------------- Phase 6: output head ----------------
            # wzat streams here (not preloaded): the collective is long done
            # by end of 5B, so this DMA no longer hits the post-collective
            # stall, and keeping it out of wpre lets the next rep's preloads
            # start mid-rep instead of after phase 6.
            with tc.tile_pool(name="p6w", bufs=2) as p6w, \
                 tc.tile_pool(name="p6ps", bufs=2, space="PSUM") as p6ps, \
                 tc.tile_pool(name="p6t", bufs=2) as p6t, \
                 tc.tile_pool(name="p6fps", bufs=1, space="PSUM") as p6fps:
                fin_ps = p6fps.tile([1, S], f32)
                for half in range(2):
                    w_t = p6w.tile([P, FC, 4 * P], fmm, tag="w")
                    nc.sync.dma_start(
                        out=w_t,
                        in_=w_in["wzat"].ap()[half].rearrange(
                            "p (c o) -> p c o", c=FC))
                    for sub in range(4):
                        ot = half * 4 + sub
                        ow = slice(sub * P, (sub + 1) * P)
                        ps = p6ps.tile([P, S], f32, tag="ps")
                        for ch in range(FC):
                            mm(ps, w_t[:, ch, ow], zatp[:, ch, :],
                               start=(ch == 0), stop=(ch == FC - 1))
                        t_sum = p6t.tile([P, S], f32, tag="tsum")
                        nc.vector.tensor_add(t_sum, ps, remap[:, ot, :])
                        ztp = p6t.tile([P, S], f32, tag="ztp")
                        nc.scalar.activation(ztp, t_sum, AF.Tanh,
                                             bias=bias_ap(8, ot), scale=1.0)
                        # zf = remap + ib*(ztp - remap)
                        d_t = p6t.tile([P, S], f32, tag="dt")
                        nc.vector.tensor_sub(d_t, ztp, remap[:, ot, :])
                        m_t = p6t.tile([P, S], f32, tag="mt")
                        nc.vector.tensor_mul(m_t, d_t, ib)
                        zf = p6t.tile([P, S], fmm, tag="zf")
                        nc.vector.tensor_add(zf, m_t, remap[:, ot, :])
                        mm(fin_ps, wfin_sb[:, ot:ot + 1], zf,
                           start=(ot == 0), stop=(ot == FC - 1))
                phat = p6t.tile([1, S], f32, tag="phat")
                nc.scalar.activation(phat, fin_ps, AF.Sigmoid, bias=0.0, scale=1.0)
                nc.sync.dma_start(out=out.ap().rearrange("s o -> o s"), in_=phat)
    nc.finalize()
    return nc


def _to_bf16(x):
    import ml_dtypes
    return np.ascontiguousarray(np.asarray(x, dtype=np.float32).astype(
        ml_dtypes.bfloat16))


def _prep_host_inputs(inputs):
    """Transpose/shuffle weights, build per-core shards (pure layout work)."""
    R = np.ascontiguousarray(inputs["R"], dtype=np.float32)
    RT_ext = np.concatenate(
        [np.zeros((D, 1), np.float32), np.ascontiguousarray(R.T)], axis=1)

    def shuf_big(wT, blocks_fc_out):
        # wT: [in_dim, out_dim] = W.T
        ind, od = wT.shape
        if blocks_fc_out:  # [FC_out, P, (2FC_in)*P]: per out-chunk of 128
            a = wT.reshape(ind // P, P, od // P, P)       # [c_in, p, c_out, o]
            a = a.transpose(2, 1, 0, 3)                   # [c_out, p, c_in, o]
            return a.reshape(od // P, P, (ind // P) * P)
        else:  # [2, P, FC_in * 512]: per out-half of 512
            a = wT.reshape(ind // P, P, 2, od // 2)       # [c_in, p, h, o]
            a = a.transpose(2, 1, 0, 3)                   # [h, p, c_in, o]
            return a.reshape(2, P, (ind // P) * (od // 2))

    w = {}
    w["wa"] = shuf_big(np.asarray(inputs["W_alpha"]).T, True)
    w["wd"] = shuf_big(np.asarray(inputs["W_delta"]).T, True)
    for nm, key in [("wz", "W_z"), ("wq", "W_q"), ("wk", "W_k"),
                    ("wv", "W_v"), ("wema", "W_EMA"), ("wf", "W_f"),
                    ("wzat", "W_z_at")]:
        w[nm] = shuf_big(np.asarray(inputs[key]).T, False)
    w["wi"] = np.asarray(inputs["W_i"]).T.reshape(FC, P).T  # [P, FC]
    w["wfin"] = np.asarray(inputs["W_final"]).T.reshape(FC, P).T
    w = {k: _to_bf16(v) for k, v in w.items()}

    biases = np.zeros((10, D), np.float32)
    biases[0] = inputs["b_alpha"]
    biases[1] = inputs["b_delta"]
    biases[2] = inputs["b_z"]
    biases[3] = inputs["b_q"] * ATT_SCALE
    biases[4] = inputs["b_k"]
    biases[5] = inputs["b_v"]
    biases[6] = inputs["b_EMA"]
    biases[7] = inputs["b_f"]
    biases[8] = inputs["b_z_at"]
    biases[9, 0] = np.float32(inputs["b_i"][0])

    in_maps = []
    for c in range(NCORES):
        rt_c = RT_ext[:, c * S:c * S + S + 1]              # [D, S+1]
        rt_c = rt_c.reshape(FC, P, S + 1).transpose(1, 0, 2).reshape(
            P, FC * (S + 1))                               # [P, FC*(S+1)]
        m = {"rt": _to_bf16(rt_c), "biases": biases}
        m.update(w)
        in_maps.append(m)
    return in_maps


def kernel(**inputs):
    from concourse.bass_utils import run_bass_kernel_spmd

    if "nc" not in _CACHE:
        _CACHE["nc"] = _build_bass()
    nc = _CACHE["nc"]
    in_maps = _prep_host_inputs(inputs)
    res = run_bass_kernel_spmd(nc, in_maps, core_ids=list(range(NCORES)))
    outs = [res.results[c]["out"] for c in range(NCORES)]
    return np.concatenate(outs, axis=0).astype(np.float32)


# revision 20
# speedup vs baseline: 1.4270x; 1.4270x over previous
"""MEGADecoder forward pass as a Bass/Tile kernel on 8 TRN2 NeuronCores. v2.

Sharding: sequence-parallel, 512 rows/core, params replicated. Single-head
full attention via AllGather of (K.T, V) in bf16, split into two collectives
(K.T first, V second) so gathers overlap the Q / R_EMA' / f / i projections.

All matmul operands are bf16 (full PE rate, half the DMA/collective bytes of
f32r); PSUM accumulation is f32. Weights are pre-shuffled on the host so every
weight DMA is contiguous per partition line.

Layout: activations feature-major [128 part x FC chunks x 512 seq free]; every
GEMM is a chain of 128x128x512 PE matmuls, no transposes. Softmax runs without
max-subtraction; denominator from a ones-vector matmul accumulated over
j-chunks.
"""

import numpy as np

SEQ = 4096
D = 1024
NCORES = 8
S = SEQ // NCORES  # 512 rows per core
P = 128
FC = D // P  # 8 feature chunks
NJ = SEQ // P  # 32 j-chunks
ATT_SCALE = 1.0 / float(np.sqrt(np.float32(D)))

KT_ELEMS = D * S
V_ELEMS = S * D

_CACHE = {}


def _build_bass(with_collective=True, reps=1, split_kv=False):
    import concourse.bacc as bacc
    import concourse.tile as tile
    import concourse.mybir as mybir

    f32 = mybir.dt.float32
    bf16 = mybir.dt.bfloat16
    fmm = bf16
    AF = mybir.ActivationFunctionType

    nc = bacc.Bacc(None, target_bir_lowering=False, num_devices=NCORES)
    mm = nc.tensor.matmul

    # ---- DRAM I/O (host pre-shuffled layouts) ----
    # rt: [P, FC*(S+1)] so the load is one contiguous DMA per partition line.
    rt = nc.dram_tensor("rt", [P, FC * (S + 1)], fmm, kind="ExternalInput")
    # big weights as [halves, P, 2FC or FC, 512]-style blocks, contiguous.
    w_in = {}
    for name, shp in [
        ("wa", [FC, P, 2 * FC * P]),   # per out-chunk blocks
        ("wd", [FC, P, 2 * FC * P]),
        ("wz", [2, P, FC * 4 * P]),    # per half blocks
        ("wq", [2, P, FC * 4 * P]),
        ("wk", [2, P, FC * 4 * P]),
        ("wv", [2, P, FC * 4 * P]),
        ("wema", [2, P, FC * 4 * P]),
        ("wf", [2, P, FC * 4 * P]),
        ("wzat", [2, P, FC * 4 * P]),
        ("wi", [P, FC]),
        ("wfin", [P, FC]),
    ]:
        w_in[name] = nc.dram_tensor(name, shp, fmm, kind="ExternalInput")
    # biases packed [10, D]: rows alpha,delta,z,q(pre-scaled),k,v,ema,f,zat,i
    biases = nc.dram_tensor("biases", [10, D], f32, kind="ExternalInput")
    out = nc.dram_tensor("out", [S, 1], f32, kind="ExternalOutput")

    with tile.TileContext(nc) as tc, \
         tc.tile_pool(name="consts", bufs=1) as consts, \
         tc.tile_pool(name="dram", bufs=1, space="DRAM") as dram, \
         tc.tile_pool(name="bigd", bufs=2) as bigd, \
         tc.tile_pool(name="big", bufs=1) as big:

        bsb = consts.tile([P, 10, FC], f32)
        nc.sync.dma_start(out=bsb, in_=biases.ap().rearrange("b (c p) -> p b c", p=P))
        ones_f32 = consts.tile([P, 1], f32)
        nc.vector.memset(ones_f32, 1.0)
        ones_col = consts.tile([P, 1], fmm)
        nc.scalar.copy(ones_col, ones_f32)
        bv_b = consts.tile([P, D], f32)
        nc.sync.dma_start(out=bv_b, in_=biases.ap()[5:6, :].partition_broadcast(P))

        def bias_ap(row, chunk):
            return bsb[:, row, chunk:chunk + 1]

        for _rep in range(reps):
          # Per-rep DRAM buffers: a Shared (collective-output) tensor may only
          # be written by a single instruction, so reps can't share them.
          row_bounce = dram.tile([2, S], f32, tag=f"rb{_rep}")
          if split_kv:
              kt_in = dram.tile([KT_ELEMS], fmm, tag=f"kti{_rep}")
              kt_out = dram.tile([NCORES, KT_ELEMS], fmm, addr_space="Shared",
                                 tag=f"kto{_rep}")
              v_in = dram.tile([V_ELEMS], fmm, tag=f"vi{_rep}")
              v_out = dram.tile([NCORES, V_ELEMS], fmm, addr_space="Shared",
                                tag=f"vo{_rep}")
          else:
              kv_in = dram.tile([KT_ELEMS + V_ELEMS], fmm, tag=f"kvi{_rep}")
              kv_out = dram.tile([NCORES, KT_ELEMS + V_ELEMS], fmm,
                                 addr_space="Shared", tag=f"kvo{_rep}")
              kt_in = kv_in[0:KT_ELEMS]
              v_in = kv_in[KT_ELEMS:]
          # Preload every weight consumed AFTER the collectives are issued: a
          # DMA issued after a collective in program order stalls until the
          # collective completes, so post-collective phases must not DMA.
          with tc.tile_pool(name="wpre", bufs=1) as wpre_pool:
            wpre = {}
            for wname in ("wq", "wema", "wf"):
                for half in range(2):
                    t = wpre_pool.tile([P, FC, 4 * P], fmm,
                                       tag=f"{wname}{half}")
                    nc.sync.dma_start(
                        out=t,
                        in_=w_in[wname].ap()[half].rearrange(
                            "p (c o) -> p c o", c=FC))
                    wpre[(wname, half)] = t
            wi_sb = wpre_pool.tile([P, FC], fmm, tag="wi")
            nc.sync.dma_start(out=wi_sb, in_=w_in["wi"].ap())
            wfin_sb = wpre_pool.tile([P, FC], fmm, tag="wfin")
            nc.sync.dma_start(out=wfin_sb, in_=w_in["wfin"].ap())
            # whole-kernel resident activations
            rema = bigd.tile([P, FC, S], fmm, tag="rema")
            z = bigd.tile([P, FC, S], fmm, tag="z")
            qT = big.tile([P, FC, S], fmm, tag="qT")
            remap = big.tile([P, FC, S], fmm, tag="remap")
            fT = big.tile([P, FC, S], f32, tag="fT")
            zatp = big.tile([P, FC, S], fmm, tag="zatp")
            ib = big.tile([P, S], f32, tag="ib")


            # ---------------- Phase 1: R_EMA ----------------
            with tc.tile_pool(name="p_rt", bufs=1) as p_rt, \
                 tc.tile_pool(name="p1w", bufs=3) as p1w, \
                 tc.tile_pool(name="p1ps", bufs=2, space="PSUM") as p1ps, \
                 tc.tile_pool(name="p1t", bufs=2) as p1t:
                rt_sb = p_rt.tile([P, FC, S + 1], fmm)
                nc.sync.dma_start(
                    out=rt_sb, in_=rt.ap().rearrange("p (c s) -> p c s", c=FC))
                for ot in range(FC):
                    wa_t = p1w.tile([P, 2 * FC, P], fmm, tag="wa")
                    nc.sync.dma_start(
                        out=wa_t,
                        in_=w_in["wa"].ap()[ot].rearrange("p (c o) -> p c o", o=P))
                    wd_t = p1w.tile([P, 2 * FC, P], fmm, tag="wd")
                    nc.sync.dma_start(
                        out=wd_t,
                        in_=w_in["wd"].ap()[ot].rearrange("p (c o) -> p c o", o=P))
                    ps_a = p1ps.tile([P, S], f32, tag="psa")
                    ps_d = p1ps.tile([P, S], f32, tag="psd")
                    for ch in range(FC):
                        mm(ps_a, wa_t[:, ch, :], rt_sb[:, ch, 0:S],
                           start=(ch == 0), stop=False)
                        mm(ps_d, wd_t[:, ch, :], rt_sb[:, ch, 0:S],
                           start=(ch == 0), stop=False)
                    for ch in range(FC):
                        mm(ps_a, wa_t[:, FC + ch, :], rt_sb[:, ch, 1:S + 1],
                           start=False, stop=(ch == FC - 1))
                        mm(ps_d, wd_t[:, FC + ch, :], rt_sb[:, ch, 1:S + 1],
                           start=False, stop=(ch == FC - 1))
                    alpha_t = p1t.tile([P, S], f32, tag="alpha")
                    nc.scalar.activation(alpha_t, ps_a, AF.Tanh,
                                         bias=bias_ap(0, ot), scale=1.0)
                    delta_t = p1t.tile([P, S], f32, tag="delta")
                    nc.scalar.activation(delta_t, ps_d, AF.Tanh,
                                         bias=bias_ap(1, ot), scale=1.0)
                    # rema = t1 + alpha*(r_t - t1), t1 = delta*r_prev
                    t1 = p1t.tile([P, S], f32, tag="t1")
                    nc.vector.tensor_mul(t1, delta_t, rt_sb[:, ot, 0:S])
                    t2 = p1t.tile([P, S], f32, tag="t2")
                    nc.vector.tensor_sub(t2, rt_sb[:, ot, 1:S + 1], t1)
                    t3 = p1t.tile([P, S], f32, tag="t3")
                    nc.vector.tensor_mul(t3, alpha_t, t2)
                    nc.vector.tensor_add(rema[:, ot, :], t3, t1)

            # ---------------- Phase 2: Z, K.T, V (collectives), Q.T ----------
            with tc.tile_pool(name="p2w", bufs=3) as p2w, \
                 tc.tile_pool(name="p2ps", bufs=4, space="PSUM") as p2ps, \
                 tc.tile_pool(name="p_kv", bufs=1) as p_kv:
                def proj(w_name, rhs_src, out_tile, func, bias_row, scale=1.0,
                         chunk_dma=None, preloaded=False):
                    for half in range(2):
                        if preloaded:
                            w_t = wpre[(w_name, half)]
                        else:
                            w_t = p2w.tile([P, FC, 4 * P], fmm, tag="w")
                            nc.sync.dma_start(
                                out=w_t,
                                in_=w_in[w_name].ap()[half].rearrange(
                                    "p (c o) -> p c o", c=FC))
                        for sub in range(4):
                            ot = half * 4 + sub
                            ow = slice(sub * P, (sub + 1) * P)
                            ps = p2ps.tile([P, S], f32, tag="ps")
                            for ch in range(FC):
                                mm(ps, w_t[:, ch, ow], rhs_src[:, ch, :],
                                   start=(ch == 0), stop=(ch == FC - 1))
                            nc.scalar.activation(out_tile[:, ot, :], ps, func,
                                                 bias=bias_ap(bias_row, ot),
                                                 scale=scale)
                            if chunk_dma is not None:
                                chunk_dma(ot, out_tile)

                proj("wz", rema, z, AF.Silu, 2)

                # K.T chunks: DMA each chunk to kt_in as soon as it's ready.
                ktS = p_kv.tile([P, FC, S], fmm)

                def kt_dma(ot, out_tile):
                    with tc.high_priority():
                        nc.sync.dma_start(
                            out=kt_in[ot * P * S:(ot + 1) * P * S].rearrange(
                                "(p s) -> p s", p=P),
                            in_=out_tile[:, ot, :])

                proj("wk", z, ktS, AF.Identity, 4, chunk_dma=kt_dma)

                # V seq-major: V[s, o] = sum_d Z.T[d, s] Wv.T[d, o] (+ bv)
                for half in range(2):
                    osl = slice(half * 4 * P, (half + 1) * 4 * P)
                    wv_t = p2w.tile([P, FC, 4 * P], fmm, tag="w")
                    nc.sync.dma_start(
                        out=wv_t,
                        in_=w_in["wv"].ap()[half].rearrange("p (c o) -> p c o", c=FC))
                    for st in range(4):
                        ssl = slice(st * P, (st + 1) * P)
                        ps = p2ps.tile([P, 4 * P], f32, tag="ps")
                        for ch in range(FC):
                            mm(ps, z[:, ch, ssl], wv_t[:, ch, :],
                               start=(ch == 0), stop=(ch == FC - 1))
                        v_sb = p_kv.tile([P, 4 * P], fmm, tag="vsb")
                        nc.vector.tensor_add(v_sb, ps, bv_b[:, osl])
                        with tc.high_priority():
                            nc.sync.dma_start(
                                out=v_in[st * P * D:(st + 1) * P * D].rearrange(
                                    "(p o) -> p o", p=P)[:, osl],
                                in_=v_sb)

                if with_collective:
                    if split_kv:
                        with tc.high_priority():
                            nc.gpsimd.collective_compute(
                                "AllGather", mybir.AluOpType.bypass,
                                replica_groups=[list(range(NCORES))],
                                ins=[kt_in[:].opt()], outs=[kt_out[:].opt()],
                            )
                            nc.gpsimd.collective_compute(
                                "AllGather", mybir.AluOpType.bypass,
                                replica_groups=[list(range(NCORES))],
                                ins=[v_in[:].opt()], outs=[v_out[:].opt()],
                            )
                    else:
                        with tc.high_priority():
                            nc.gpsimd.collective_compute(
                                "AllGather", mybir.AluOpType.bypass,
                                replica_groups=[list(range(NCORES))],
                                ins=[kv_in[:].opt()], outs=[kv_out[:].opt()],
                            )

                # Q.T after the gathers are issued — overlaps them
                # (weights preloaded, so no post-collective DMA stall).
                proj("wq", z, qT, AF.Identity, 3, scale=ATT_SCALE,
                     preloaded=True)

            # ---------------- Phase 4: R_EMA', f, i ----------------
            with tc.tile_pool(name="p4ps", bufs=2, space="PSUM") as p4ps, \
                 tc.tile_pool(name="p4psi", bufs=1, space="PSUM") as p4psi, \
                 tc.tile_pool(name="p4t", bufs=1) as p4t:
                def proj4(w_name, rhs_src, out_tile, func, bias_row):
                    for half in range(2):
                        w_t = wpre[(w_name, half)]
                        for sub in range(4):
                            ot = half * 4 + sub
                            ow = slice(sub * P, (sub + 1) * P)
                            ps = p4ps.tile([P, S], f32, tag="ps")
                            for ch in range(FC):
                                mm(ps, w_t[:, ch, ow], rhs_src[:, ch, :],
                                   start=(ch == 0), stop=(ch == FC - 1))
                            nc.scalar.activation(out_tile[:, ot, :], ps, func,
                                                 bias=bias_ap(bias_row, ot),
                                                 scale=1.0)

                proj4("wema", rema, remap, AF.Identity, 6)
                proj4("wf", remap, fT, AF.Sigmoid, 7)

                ps_i = p4psi.tile([1, S], f32)
                for ch in range(FC):
                    mm(ps_i, wi_sb[:, ch:ch + 1], rema[:, ch, :],
                       start=(ch == 0), stop=(ch == FC - 1))
                i_row = p4t.tile([1, S], f32, tag="irow")
                nc.scalar.activation(i_row, ps_i, AF.Tanh,
                                     bias=bsb[0:1, 9, 0:1], scale=1.0)
                nc.sync.dma_start(out=row_bounce[0:1, :], in_=i_row)
                nc.sync.dma_start(out=ib,
                                  in_=row_bounce[0:1, :].partition_broadcast(P))


            # ---------------- Phase 5: attention ----------------
            with tc.tile_pool(name="p_pt", bufs=1) as p_pt, \
                 tc.tile_pool(name="p_rl", bufs=1) as p_rl:
                pt = p_pt.tile([P, NJ, S], fmm)
                rl_b = p_rl.tile([P, S], f32, tag="rlb")
                # --- 5A: scores.T + exp + denominator ---
                with tc.tile_pool(name="p5w", bufs=2) as p5w, \
                     tc.tile_pool(name="p5ps", bufs=2, space="PSUM") as p5ps, \
                     tc.tile_pool(name="p5lps", bufs=1, space="PSUM") as p5lps:
                    l_ps = p5lps.tile([1, S], f32)
                    for r in range(NCORES):
                        if split_kv:
                            kt_src = kt_out[r] if with_collective else kt_in[:]
                        else:
                            kt_src = (kv_out[r, 0:KT_ELEMS] if with_collective
                                      else kt_in[:])
                        kt_r = p5w.tile([P, FC, S], fmm, tag="ktr")
                        nc.sync.dma_start(
                            out=kt_r,
                            in_=kt_src.rearrange("(c p s) -> p c s", p=P, s=S))
                        for cl in range(4):
                            jc = r * 4 + cl
                            jsl = slice(cl * P, (cl + 1) * P)
                            s_ps = p5ps.tile([P, S], f32, tag="sps")
                            for ch in range(FC):
                                mm(s_ps, kt_r[:, ch, jsl], qT[:, ch, :],
                                   start=(ch == 0), stop=(ch == FC - 1))
                            nc.scalar.activation(pt[:, jc, :], s_ps, AF.Exp,
                                                 bias=0.0, scale=1.0)
                            mm(l_ps, ones_col, pt[:, jc, :],
                               start=(jc == 0), stop=(jc == NJ - 1))
                    l_row = p_rl.tile([1, S], f32, tag="lrow")
                    nc.vector.reciprocal(l_row, l_ps)
                    nc.sync.dma_start(out=row_bounce[1:2, :], in_=l_row)
                    nc.sync.dma_start(
                        out=rl_b, in_=row_bounce[1:2, :].partition_broadcast(P))

                # --- 5B: Zat.T ---
                with tc.tile_pool(name="p5v", bufs=6) as p5v, \
                     tc.tile_pool(name="pvps", bufs=1, space="PSUM") as pvps:
                    zat_ps = []
                    for i in range(FC):
                        zp = pvps.tile([P, S], f32, tag=f"zat{i}", name=f"zat{i}")
                        zat_ps.append(zp)
                    for jc in range(NJ):
                        r, cl = jc // 4, jc % 4
                        if split_kv:
                            v_src = v_out[r] if with_collective else v_in[:]
                        else:
                            v_src = (kv_out[r, KT_ELEMS:] if with_collective
                                     else kv_in[KT_ELEMS:])
                        v_ch = p5v.tile([P, D], fmm, tag="vch")
                        nc.sync.dma_start(
                            out=v_ch,
                            in_=v_src.rearrange("(t p o) -> t p o", p=P, o=D)[cl])
                        for ot in range(FC):
                            mm(zat_ps[ot], v_ch[:, ot * P:(ot + 1) * P],
                               pt[:, jc, :],
                               start=(jc == 0), stop=(jc == NJ - 1))
                    # zatp = f * (zat/l)
                    for ot in range(FC):
                        nc.vector.tensor_mul(zat_ps[ot], zat_ps[ot], rl_b)
                        nc.vector.tensor_mul(zatp[:, ot, :], zat_ps[ot],
                                             fT[:, ot, :])

            # ---------------- Phase 6: output head ----------------
            # wzat streams here (not preloaded): the collective is long done
            # by end of 5B, so this DMA no longer hits the post-collective
            # stall, and keeping it out of wpre lets the next rep's preloads
            # start mid-rep instead of after phase 6.
            with tc.tile_pool(name="p6w", bufs=2) as p6w, \
                 tc.tile_pool(name="p6ps", bufs=2, space="PSUM") as p6ps, \
                 tc.tile_pool(name="p6t", bufs=2) as p6t, \
                 tc.tile_pool(name="p6fps", bufs=1, space="PSUM") as p6fps:
                fin_ps = p6fps.tile([1, S], f32)
                for half in range(2):
                    w_t = p6w.tile([P, FC, 4 * P], fmm, tag="w")
                    nc.sync.dma_start(
                        out=w_t,
                        in_=w_in["wzat"].ap()[half].rearrange(
                            "p (c o) -> p c o", c=FC))
                    for sub in range(4):
                        ot = half * 4 + sub
                        ow = slice(sub * P, (sub + 1) * P)
                        ps = p6ps.tile([P, S], f32, tag="ps")
                        for ch in range(FC):
                            mm(ps, w_t[:, ch, ow], zatp[:, ch, :],
                               start=(ch == 0), stop=(ch == FC - 1))
                        t_sum = p6t.tile([P, S], f32, tag="tsum")
                        nc.vector.tensor_add(t_sum, ps, remap[:, ot, :])
                        ztp = p6t.tile([P, S], f32, tag="ztp")
                        nc.scalar.activation(ztp, t_sum, AF.Tanh,
                                             bias=bias_ap(8, ot), scale=1.0)
                        # zf = remap + ib*(ztp - remap)
                        d_t = p6t.tile([P, S], f32, tag="dt")
                        nc.vector.tensor_sub(d_t, ztp, remap[:, ot, :])
                        m_t = p6t.tile([P, S], f32, tag="mt")
                        nc.vector.tensor_mul(m_t, d_t, ib)
                        zf = p6t.tile([P, S], fmm, tag="zf")
                        nc.vector.tensor_add(zf, m_t, remap[:, ot, :])
                        mm(fin_ps, wfin_sb[:, ot:ot + 1], zf,
                           start=(ot == 0), stop=(ot == FC - 1))
                phat = p6t.tile([1, S], f32, tag="phat")
                nc.scalar.activation(phat, fin_ps, AF.Sigmoid, bias=0.0, scale=1.0)
                nc.sync.dma_start(out=out.ap().rearrange("s o -> o s"), in_=phat)
    nc.finalize()
    return nc


def _to_bf16(x):
    import ml_dtypes
    return np.ascontiguousarray(np.asarray(x, dtype=np.float32).astype(
        ml_dtypes.bfloat16))


def _prep_host_inputs(inputs):
    """Transpose/shuffle weights, build per-core shards (pure layout work)."""
    R = np.ascontiguousarray(inputs["R"], dtype=np.float32)
    RT_ext = np.concatenate(
        [np.zeros((D, 1), np.float32), np.ascontiguousarray(R.T)], axis=1)

    def shuf_big(wT, blocks_fc_out):
        # wT: [in_dim, out_dim] = W.T
        ind, od = wT.shape
        if blocks_fc_out:  # [FC_out, P, (2FC_in)*P]: per out-chunk of 128
            a = wT.reshape(ind // P, P, od // P, P)       # [c_in, p, c_out, o]
            a = a.transpose(2, 1, 0, 3)                   # [c_out, p, c_in, o]
            return a.reshape(od // P, P, (ind // P) * P)
        else:  # [2, P, FC_in * 512]: per out-half of 512
            a = wT.reshape(ind // P, P, 2, od // 2)       # [c_in, p, h, o]
            a = a.transpose(2, 1, 0, 3)                   # [h, p, c_in, o]
            return a.reshape(2, P, (ind // P) * (od // 2))

    w = {}
    w["wa"] = shuf_big(np.asarray(inputs["W_alpha"]).T, True)
    w["wd"] = shuf_big(np.asarray(inputs["W_delta"]).T, True)
    for nm, key in [("wz", "W_z"), ("wq", "W_q"), ("wk", "W_k"),
                    ("wv", "W_v"), ("wema", "W_EMA"), ("wf", "W_f"),
                    ("wzat", "W_z_at")]:
        w[nm] = shuf_big(np.asarray(inputs[key]).T, False)
    w["wi"] = np.asarray(inputs["W_i"]).T.reshape(FC, P).T  # [P, FC]
    w["wfin"] = np.asarray(inputs["W_final"]).T.reshape(FC, P).T
    w = {k: _to_bf16(v) for k, v in w.items()}

    biases = np.zeros((10, D), np.float32)
    biases[0] = inputs["b_alpha"]
    biases[1] = inputs["b_delta"]
    biases[2] = inputs["b_z"]
    biases[3] = inputs["b_q"] * ATT_SCALE
    biases[4] = inputs["b_k"]
    biases[5] = inputs["b_v"]
    biases[6] = inputs["b_EMA"]
    biases[7] = inputs["b_f"]
    biases[8] = inputs["b_z_at"]
    biases[9, 0] = np.float32(inputs["b_i"][0])

    in_maps = []
    for c in range(NCORES):
        rt_c = RT_ext[:, c * S:c * S + S + 1]              # [D, S+1]
        rt_c = rt_c.reshape(FC, P, S + 1).transpose(1, 0, 2).reshape(
            P, FC * (S + 1))                               # [P, FC*(S+1)]
        m = {"rt": _to_bf16(rt_c), "biases": biases}
        m.update(w)
        in_maps.append(m)
    return in_maps


def kernel(**inputs):
    from concourse.bass_utils import run_bass_kernel_spmd

    if "nc" not in _CACHE:
        _CACHE["nc"] = _build_bass()
    nc = _CACHE["nc"]
    in_maps = _prep_host_inputs(inputs)
    res = run_bass_kernel_spmd(nc, in_maps, core_ids=list(range(NCORES)))
    outs = [res.results[c]["out"] for c in range(NCORES)]
    return np.concatenate(outs, axis=0).astype(np.float32)


# revision 25
# speedup vs baseline: 1.5985x; 1.1202x over previous
"""MEGADecoder forward pass as a Bass/Tile kernel on 8 TRN2 NeuronCores. v2.

Sharding: sequence-parallel, 512 rows/core, params replicated. Single-head
full attention via AllGather of (K.T, V) in bf16, split into two collectives
(K.T first, V second) so gathers overlap the Q / R_EMA' / f / i projections.

All matmul operands are bf16 (full PE rate, half the DMA/collective bytes of
f32r); PSUM accumulation is f32. Weights are pre-shuffled on the host so every
weight DMA is contiguous per partition line.

Layout: activations feature-major [128 part x FC chunks x 512 seq free]; every
GEMM is a chain of 128x128x512 PE matmuls, no transposes. Softmax runs without
max-subtraction; denominator from a ones-vector matmul accumulated over
j-chunks.
"""

import numpy as np

SEQ = 4096
D = 1024
NCORES = 8
S = SEQ // NCORES  # 512 rows per core
P = 128
FC = D // P  # 8 feature chunks
NJ = SEQ // P  # 32 j-chunks
ATT_SCALE = 1.0 / float(np.sqrt(np.float32(D)))

KT_ELEMS = D * S
V_ELEMS = S * D

_CACHE = {}


def _build_bass(with_collective=True, reps=1, split_kv=False):
    import concourse.bacc as bacc
    import concourse.tile as tile
    import concourse.mybir as mybir

    f32 = mybir.dt.float32
    bf16 = mybir.dt.bfloat16
    fmm = bf16
    AF = mybir.ActivationFunctionType

    nc = bacc.Bacc(None, target_bir_lowering=False, num_devices=NCORES)
    mm = nc.tensor.matmul

    # ---- DRAM I/O (host pre-shuffled layouts) ----
    # rt: [P, FC*(S+1)] so the load is one contiguous DMA per partition line.
    rt = nc.dram_tensor("rt", [P, FC * (S + 1)], fmm, kind="ExternalInput")
    # big weights as [halves, P, 2FC or FC, 512]-style blocks, contiguous.
    w_in = {}
    for name, shp in [
        ("wa", [FC, P, 2 * FC * P]),   # per out-chunk blocks
        ("wd", [FC, P, 2 * FC * P]),
        ("wz", [2, P, FC * 4 * P]),    # per half blocks
        ("wq", [2, P, FC * 4 * P]),    # carries M = W_q.T @ W_k / sqrt(d)
        ("wv", [2, P, FC * 4 * P]),
        ("wema", [2, P, FC * 4 * P]),
        ("wf", [2, P, FC * 4 * P]),
        ("wzat", [2, P, FC * 4 * P]),
        ("wi", [P, FC]),
        ("wfin", [P, FC]),
    ]:
        w_in[name] = nc.dram_tensor(name, shp, fmm, kind="ExternalInput")
    # biases packed [10, D]: rows alpha,delta,z,q(pre-scaled),k,v,ema,f,zat,i
    biases = nc.dram_tensor("biases", [10, D], f32, kind="ExternalInput")
    out = nc.dram_tensor("out", [S, 1], f32, kind="ExternalOutput")

    with tile.TileContext(nc) as tc, \
         tc.tile_pool(name="consts", bufs=1) as consts, \
         tc.tile_pool(name="dram", bufs=1, space="DRAM") as dram, \
         tc.tile_pool(name="bigd", bufs=2) as bigd, \
         tc.tile_pool(name="big", bufs=1) as big:

        bsb = consts.tile([P, 10, FC], f32)
        nc.sync.dma_start(out=bsb, in_=biases.ap().rearrange("b (c p) -> p b c", p=P))
        ones_f32 = consts.tile([P, 1], f32)
        nc.vector.memset(ones_f32, 1.0)
        ones_col = consts.tile([P, 1], fmm)
        nc.scalar.copy(ones_col, ones_f32)
        bv_b = consts.tile([P, D], f32)
        nc.sync.dma_start(out=bv_b, in_=biases.ap()[5:6, :].partition_broadcast(P))

        def bias_ap(row, chunk):
            return bsb[:, row, chunk:chunk + 1]

        for _rep in range(reps):
          # Per-rep DRAM buffers: a Shared (collective-output) tensor may only
          # be written by a single instruction, so reps can't share them.
          row_bounce = dram.tile([2, S], f32, tag=f"rb{_rep}")
          if split_kv:
              kt_in = dram.tile([KT_ELEMS], fmm, tag=f"kti{_rep}")
              kt_out = dram.tile([NCORES, KT_ELEMS], fmm, addr_space="Shared",
                                 tag=f"kto{_rep}")
              v_in = dram.tile([V_ELEMS], fmm, tag=f"vi{_rep}")
              v_out = dram.tile([NCORES, V_ELEMS], fmm, addr_space="Shared",
                                tag=f"vo{_rep}")
          else:
              kv_in = dram.tile([KT_ELEMS + V_ELEMS], fmm, tag=f"kvi{_rep}")
              kv_out = dram.tile([NCORES, KT_ELEMS + V_ELEMS], fmm,
                                 addr_space="Shared", tag=f"kvo{_rep}")
              kt_in = kv_in[0:KT_ELEMS]
              v_in = kv_in[KT_ELEMS:]
          # Preload every weight consumed AFTER the collectives are issued: a
          # DMA issued after a collective in program order stalls until the
          # collective completes, so post-collective phases must not DMA.
          with tc.tile_pool(name="wpre", bufs=1) as wpre_pool:
            wpre = {}
            for wname in ("wq", "wema", "wf"):
                for half in range(2):
                    t = wpre_pool.tile([P, FC, 4 * P], fmm,
                                       tag=f"{wname}{half}")
                    nc.sync.dma_start(
                        out=t,
                        in_=w_in[wname].ap()[half].rearrange(
                            "p (c o) -> p c o", c=FC))
                    wpre[(wname, half)] = t
            wi_sb = wpre_pool.tile([P, FC], fmm, tag="wi")
            nc.sync.dma_start(out=wi_sb, in_=w_in["wi"].ap())
            wfin_sb = wpre_pool.tile([P, FC], fmm, tag="wfin")
            nc.sync.dma_start(out=wfin_sb, in_=w_in["wfin"].ap())
            # whole-kernel resident activations
            rema = bigd.tile([P, FC, S], fmm, tag="rema")
            z = bigd.tile([P, FC, S], fmm, tag="z")
            qT = big.tile([P, FC, S], fmm, tag="qT")
            remap = big.tile([P, FC, S], fmm, tag="remap")
            fT = big.tile([P, FC, S], f32, tag="fT")
            zatp = big.tile([P, FC, S], fmm, tag="zatp")
            ib = big.tile([P, S], f32, tag="ib")


            # ---------------- Phase 1: R_EMA ----------------
            with tc.tile_pool(name="p_rt", bufs=1) as p_rt, \
                 tc.tile_pool(name="p1w", bufs=3) as p1w, \
                 tc.tile_pool(name="p1ps", bufs=2, space="PSUM") as p1ps, \
                 tc.tile_pool(name="p1t", bufs=2) as p1t:
                rt_sb = p_rt.tile([P, FC, S + 1], fmm)
                nc.sync.dma_start(
                    out=rt_sb, in_=rt.ap().rearrange("p (c s) -> p c s", c=FC))
                for ot in range(FC):
                    wa_t = p1w.tile([P, 2 * FC, P], fmm, tag="wa")
                    nc.sync.dma_start(
                        out=wa_t,
                        in_=w_in["wa"].ap()[ot].rearrange("p (c o) -> p c o", o=P))
                    wd_t = p1w.tile([P, 2 * FC, P], fmm, tag="wd")
                    nc.sync.dma_start(
                        out=wd_t,
                        in_=w_in["wd"].ap()[ot].rearrange("p (c o) -> p c o", o=P))
                    ps_a = p1ps.tile([P, S], f32, tag="psa")
                    ps_d = p1ps.tile([P, S], f32, tag="psd")
                    for ch in range(FC):
                        mm(ps_a, wa_t[:, ch, :], rt_sb[:, ch, 0:S],
                           start=(ch == 0), stop=False)
                        mm(ps_d, wd_t[:, ch, :], rt_sb[:, ch, 0:S],
                           start=(ch == 0), stop=False)
                    for ch in range(FC):
                        mm(ps_a, wa_t[:, FC + ch, :], rt_sb[:, ch, 1:S + 1],
                           start=False, stop=(ch == FC - 1))
                        mm(ps_d, wd_t[:, FC + ch, :], rt_sb[:, ch, 1:S + 1],
                           start=False, stop=(ch == FC - 1))
                    alpha_t = p1t.tile([P, S], f32, tag="alpha")
                    nc.scalar.activation(alpha_t, ps_a, AF.Tanh,
                                         bias=bias_ap(0, ot), scale=1.0)
                    delta_t = p1t.tile([P, S], f32, tag="delta")
                    nc.scalar.activation(delta_t, ps_d, AF.Tanh,
                                         bias=bias_ap(1, ot), scale=1.0)
                    # rema = t1 + alpha*(r_t - t1), t1 = delta*r_prev
                    t1 = p1t.tile([P, S], f32, tag="t1")
                    nc.vector.tensor_mul(t1, delta_t, rt_sb[:, ot, 0:S])
                    t2 = p1t.tile([P, S], f32, tag="t2")
                    nc.vector.tensor_sub(t2, rt_sb[:, ot, 1:S + 1], t1)
                    t3 = p1t.tile([P, S], f32, tag="t3")
                    nc.vector.tensor_mul(t3, alpha_t, t2)
                    nc.vector.tensor_add(rema[:, ot, :], t3, t1)

            # ---------------- Phase 2: Z, K.T, V (collectives), Q.T ----------
            with tc.tile_pool(name="p2w", bufs=3) as p2w, \
                 tc.tile_pool(name="p2ps", bufs=4, space="PSUM") as p2ps, \
                 tc.tile_pool(name="p_kv", bufs=1) as p_kv:
                def proj(w_name, rhs_src, out_tile, func, bias_row, scale=1.0,
                         chunk_dma=None, preloaded=False):
                    for half in range(2):
                        if preloaded:
                            w_t = wpre[(w_name, half)]
                        else:
                            w_t = p2w.tile([P, FC, 4 * P], fmm, tag="w")
                            nc.sync.dma_start(
                                out=w_t,
                                in_=w_in[w_name].ap()[half].rearrange(
                                    "p (c o) -> p c o", c=FC))
                        for sub in range(4):
                            ot = half * 4 + sub
                            ow = slice(sub * P, (sub + 1) * P)
                            ps = p2ps.tile([P, S], f32, tag="ps")
                            for ch in range(FC):
                                mm(ps, w_t[:, ch, ow], rhs_src[:, ch, :],
                                   start=(ch == 0), stop=(ch == FC - 1))
                            nc.scalar.activation(out_tile[:, ot, :], ps, func,
                                                 bias=bias_ap(bias_row, ot),
                                                 scale=scale)
                            if chunk_dma is not None:
                                chunk_dma(ot, out_tile)

                # K is never materialized: softmax is invariant to per-query
                # constants, so scores = (Z M) Z_full.T with M = Wq.T Wk/sqrt(d)
                # folded on the host. The gather ships Z.T chunks (same bytes
                # K.T would have been), streamed out as each chunk is ready.
                def z_dma(ot, out_tile):
                    with tc.high_priority():
                        nc.sync.dma_start(
                            out=kt_in[ot * P * S:(ot + 1) * P * S].rearrange(
                                "(p s) -> p s", p=P),
                            in_=out_tile[:, ot, :])

                proj("wz", rema, z, AF.Silu, 2, chunk_dma=z_dma)

                # V seq-major: V[s, o] = sum_d Z.T[d, s] Wv.T[d, o] (+ bv)
                for half in range(2):
                    osl = slice(half * 4 * P, (half + 1) * 4 * P)
                    wv_t = p2w.tile([P, FC, 4 * P], fmm, tag="w")
                    nc.sync.dma_start(
                        out=wv_t,
                        in_=w_in["wv"].ap()[half].rearrange("p (c o) -> p c o", c=FC))
                    for st in range(4):
                        ssl = slice(st * P, (st + 1) * P)
                        ps = p2ps.tile([P, 4 * P], f32, tag="ps")
                        for ch in range(FC):
                            mm(ps, z[:, ch, ssl], wv_t[:, ch, :],
                               start=(ch == 0), stop=(ch == FC - 1))
                        v_sb = p_kv.tile([P, 4 * P], fmm, tag="vsb")
                        nc.vector.tensor_add(v_sb, ps, bv_b[:, osl])
                        with tc.high_priority():
                            nc.sync.dma_start(
                                out=v_in[st * P * D:(st + 1) * P * D].rearrange(
                                    "(p o) -> p o", p=P)[:, osl],
                                in_=v_sb)

                if with_collective:
                    if split_kv:
                        with tc.high_priority():
                            nc.gpsimd.collective_compute(
                                "AllGather", mybir.AluOpType.bypass,
                                replica_groups=[list(range(NCORES))],
                                ins=[kt_in[:].opt()], outs=[kt_out[:].opt()],
                            )
                            nc.gpsimd.collective_compute(
                                "AllGather", mybir.AluOpType.bypass,
                                replica_groups=[list(range(NCORES))],
                                ins=[v_in[:].opt()], outs=[v_out[:].opt()],
                            )
                    else:
                        with tc.high_priority():
                            nc.gpsimd.collective_compute(
                                "AllGather", mybir.AluOpType.bypass,
                                replica_groups=[list(range(NCORES))],
                                ins=[kv_in[:].opt()], outs=[kv_out[:].opt()],
                            )

                # Q.T after the gathers are issued — overlaps them
                # (weights preloaded, so no post-collective DMA stall).
                proj("wq", z, qT, AF.Identity, 3, preloaded=True)

            # ---------------- Phase 4: R_EMA', f, i ----------------
            with tc.tile_pool(name="p4ps", bufs=2, space="PSUM") as p4ps, \
                 tc.tile_pool(name="p4psi", bufs=1, space="PSUM") as p4psi, \
                 tc.tile_pool(name="p4t", bufs=1) as p4t:
                def proj4(w_name, rhs_src, out_tile, func, bias_row):
                    for half in range(2):
                        w_t = wpre[(w_name, half)]
                        for sub in range(4):
                            ot = half * 4 + sub
                            ow = slice(sub * P, (sub + 1) * P)
                            ps = p4ps.tile([P, S], f32, tag="ps")
                            for ch in range(FC):
                                mm(ps, w_t[:, ch, ow], rhs_src[:, ch, :],
                                   start=(ch == 0), stop=(ch == FC - 1))
                            nc.scalar.activation(out_tile[:, ot, :], ps, func,
                                                 bias=bias_ap(bias_row, ot),
                                                 scale=1.0)

                proj4("wema", rema, remap, AF.Identity, 6)
                proj4("wf", remap, fT, AF.Sigmoid, 7)

                ps_i = p4psi.tile([1, S], f32)
                for ch in range(FC):
                    mm(ps_i, wi_sb[:, ch:ch + 1], rema[:, ch, :],
                       start=(ch == 0), stop=(ch == FC - 1))
                i_row = p4t.tile([1, S], f32, tag="irow")
                nc.scalar.activation(i_row, ps_i, AF.Tanh,
                                     bias=bsb[0:1, 9, 0:1], scale=1.0)
                nc.sync.dma_start(out=row_bounce[0:1, :], in_=i_row)
                nc.sync.dma_start(out=ib,
                                  in_=row_bounce[0:1, :].partition_broadcast(P))


            # ---------------- Phase 5: attention ----------------
            with tc.tile_pool(name="p_pt", bufs=1) as p_pt, \
                 tc.tile_pool(name="p_rl", bufs=1) as p_rl:
                pt = p_pt.tile([P, NJ, S], fmm)
                rl_b = p_rl.tile([P, S], f32, tag="rlb")
                # --- 5A: scores.T + exp + denominator ---
                with tc.tile_pool(name="p5w", bufs=2) as p5w, \
                     tc.tile_pool(name="p5ps", bufs=2, space="PSUM") as p5ps, \
                     tc.tile_pool(name="p5lps", bufs=1, space="PSUM") as p5lps:
                    l_ps = p5lps.tile([1, S], f32)
                    for r in range(NCORES):
                        if split_kv:
                            kt_src = kt_out[r] if with_collective else kt_in[:]
                        else:
                            kt_src = (kv_out[r, 0:KT_ELEMS] if with_collective
                                      else kt_in[:])
                        kt_r = p5w.tile([P, FC, S], fmm, tag="ktr")
                        nc.sync.dma_start(
                            out=kt_r,
                            in_=kt_src.rearrange("(c p s) -> p c s", p=P, s=S))
                        for cl in range(4):
                            jc = r * 4 + cl
                            jsl = slice(cl * P, (cl + 1) * P)
                            s_ps = p5ps.tile([P, S], f32, tag="sps")
                            for ch in range(FC):
                                mm(s_ps, kt_r[:, ch, jsl], qT[:, ch, :],
                                   start=(ch == 0), stop=(ch == FC - 1))
                            nc.scalar.activation(pt[:, jc, :], s_ps, AF.Exp,
                                                 bias=0.0, scale=1.0)
                            mm(l_ps, ones_col, pt[:, jc, :],
                               start=(jc == 0), stop=(jc == NJ - 1))
                    l_row = p_rl.tile([1, S], f32, tag="lrow")
                    nc.vector.reciprocal(l_row, l_ps)
                    nc.sync.dma_start(out=row_bounce[1:2, :], in_=l_row)
                    nc.sync.dma_start(
                        out=rl_b, in_=row_bounce[1:2, :].partition_broadcast(P))

                # --- 5B: Zat.T ---
                with tc.tile_pool(name="p5v", bufs=6) as p5v, \
                     tc.tile_pool(name="pvps", bufs=1, space="PSUM") as pvps:
                    zat_ps = []
                    for i in range(FC):
                        zp = pvps.tile([P, S], f32, tag=f"zat{i}", name=f"zat{i}")
                        zat_ps.append(zp)
                    for jc in range(NJ):
                        r, cl = jc // 4, jc % 4
                        if split_kv:
                            v_src = v_out[r] if with_collective else v_in[:]
                        else:
                            v_src = (kv_out[r, KT_ELEMS:] if with_collective
                                     else kv_in[KT_ELEMS:])
                        v_ch = p5v.tile([P, D], fmm, tag="vch")
                        nc.sync.dma_start(
                            out=v_ch,
                            in_=v_src.rearrange("(t p o) -> t p o", p=P, o=D)[cl])
                        for ot in range(FC):
                            mm(zat_ps[ot], v_ch[:, ot * P:(ot + 1) * P],
                               pt[:, jc, :],
                               start=(jc == 0), stop=(jc == NJ - 1))
                    # zatp = f * (zat/l)
                    for ot in range(FC):
                        nc.vector.tensor_mul(zat_ps[ot], zat_ps[ot], rl_b)
                        nc.vector.tensor_mul(zatp[:, ot, :], zat_ps[ot],
                                             fT[:, ot, :])

            # ---------------- Phase 6: output head ----------------
            # wzat streams here (not preloaded): the collective is long done
            # by end of 5B, so this DMA no longer hits the post-collective
            # stall, and keeping it out of wpre lets the next rep's preloads
            # start mid-rep instead of after phase 6.
            with tc.tile_pool(name="p6w", bufs=2) as p6w, \
                 tc.tile_pool(name="p6ps", bufs=2, space="PSUM") as p6ps, \
                 tc.tile_pool(name="p6t", bufs=2) as p6t, \
                 tc.tile_pool(name="p6fps", bufs=1, space="PSUM") as p6fps:
                fin_ps = p6fps.tile([1, S], f32)
                for half in range(2):
                    w_t = p6w.tile([P, FC, 4 * P], fmm, tag="w")
                    nc.sync.dma_start(
                        out=w_t,
                        in_=w_in["wzat"].ap()[half].rearrange(
                            "p (c o) -> p c o", c=FC))
                    for sub in range(4):
                        ot = half * 4 + sub
                        ow = slice(sub * P, (sub + 1) * P)
                        ps = p6ps.tile([P, S], f32, tag="ps")
                        for ch in range(FC):
                            mm(ps, w_t[:, ch, ow], zatp[:, ch, :],
                               start=(ch == 0), stop=(ch == FC - 1))
                        t_sum = p6t.tile([P, S], f32, tag="tsum")
                        nc.vector.tensor_add(t_sum, ps, remap[:, ot, :])
                        ztp = p6t.tile([P, S], f32, tag="ztp")
                        nc.scalar.activation(ztp, t_sum, AF.Tanh,
                                             bias=bias_ap(8, ot), scale=1.0)
                        # zf = remap + ib*(ztp - remap)
                        d_t = p6t.tile([P, S], f32, tag="dt")
                        nc.vector.tensor_sub(d_t, ztp, remap[:, ot, :])
                        m_t = p6t.tile([P, S], f32, tag="mt")
                        nc.vector.tensor_mul(m_t, d_t, ib)
                        zf = p6t.tile([P, S], fmm, tag="zf")
                        nc.vector.tensor_add(zf, m_t, remap[:, ot, :])
                        mm(fin_ps, wfin_sb[:, ot:ot + 1], zf,
                           start=(ot == 0), stop=(ot == FC - 1))
                phat = p6t.tile([1, S], f32, tag="phat")
                nc.scalar.activation(phat, fin_ps, AF.Sigmoid, bias=0.0, scale=1.0)
                nc.sync.dma_start(out=out.ap().rearrange("s o -> o s"), in_=phat)
    nc.finalize()
    return nc


def _to_bf16(x):
    import ml_dtypes
    return np.ascontiguousarray(np.asarray(x, dtype=np.float32).astype(
        ml_dtypes.bfloat16))


def _prep_host_inputs(inputs):
    """Transpose/shuffle weights, build per-core shards (pure layout work)."""
    R = np.ascontiguousarray(inputs["R"], dtype=np.float32)
    RT_ext = np.concatenate(
        [np.zeros((D, 1), np.float32), np.ascontiguousarray(R.T)], axis=1)

    def shuf_big(wT, blocks_fc_out):
        # wT: [in_dim, out_dim] = W.T
        ind, od = wT.shape
        if blocks_fc_out:  # [FC_out, P, (2FC_in)*P]: per out-chunk of 128
            a = wT.reshape(ind // P, P, od // P, P)       # [c_in, p, c_out, o]
            a = a.transpose(2, 1, 0, 3)                   # [c_out, p, c_in, o]
            return a.reshape(od // P, P, (ind // P) * P)
        else:  # [2, P, FC_in * 512]: per out-half of 512
            a = wT.reshape(ind // P, P, 2, od // 2)       # [c_in, p, h, o]
            a = a.transpose(2, 1, 0, 3)                   # [h, p, c_in, o]
            return a.reshape(2, P, (ind // P) * (od // 2))

    w = {}
    w["wa"] = shuf_big(np.asarray(inputs["W_alpha"]).T, True)
    w["wd"] = shuf_big(np.asarray(inputs["W_delta"]).T, True)
    for nm, key in [("wz", "W_z"), ("wv", "W_v"), ("wema", "W_EMA"),
                    ("wf", "W_f"), ("wzat", "W_z_at")]:
        w[nm] = shuf_big(np.asarray(inputs[key]).T, False)
    # folded attention matrix: scores = (Z M + bg) Z_full.T, K never built
    M_att = (np.asarray(inputs["W_q"], np.float32).T
             @ np.asarray(inputs["W_k"], np.float32)) * ATT_SCALE
    w["wq"] = shuf_big(M_att, False)
    w["wi"] = np.asarray(inputs["W_i"]).T.reshape(FC, P).T  # [P, FC]
    w["wfin"] = np.asarray(inputs["W_final"]).T.reshape(FC, P).T
    w = {k: _to_bf16(v) for k, v in w.items()}

    biases = np.zeros((10, D), np.float32)
    biases[0] = inputs["b_alpha"]
    biases[1] = inputs["b_delta"]
    biases[2] = inputs["b_z"]
    # bg = b_q @ W_k * scale; the b_k term is a per-query constant in the
    # scores and cancels exactly in softmax.
    biases[3] = (np.asarray(inputs["b_q"], np.float32)
                 @ np.asarray(inputs["W_k"], np.float32)) * ATT_SCALE
    biases[5] = inputs["b_v"]
    biases[6] = inputs["b_EMA"]
    biases[7] = inputs["b_f"]
    biases[8] = inputs["b_z_at"]
    biases[9, 0] = np.float32(inputs["b_i"][0])

    in_maps = []
    for c in range(NCORES):
        rt_c = RT_ext[:, c * S:c * S + S + 1]              # [D, S+1]
        rt_c = rt_c.reshape(FC, P, S + 1).transpose(1, 0, 2).reshape(
            P, FC * (S + 1))                               # [P, FC*(S+1)]
        m = {"rt": _to_bf16(rt_c), "biases": biases}
        m.update(w)
        in_maps.append(m)
    return in_maps


def kernel(**inputs):
    from concourse.bass_utils import run_bass_kernel_spmd

    if "nc" not in _CACHE:
        _CACHE["nc"] = _build_bass()
    nc = _CACHE["nc"]
    in_maps = _prep_host_inputs(inputs)
    res = run_bass_kernel_spmd(nc, in_maps, core_ids=list(range(NCORES)))
    outs = [res.results[c]["out"] for c in range(NCORES)]
    return np.concatenate(outs, axis=0).astype(np.float32)
